# revision 1
# baseline (speedup 1.0000x reference)
"""Causal-mask multi-head attention (B=2, S=2048, D=1024, H=16) on 8 trn2
NeuronCores.

Sharding: core c = 4*b + g handles batch b and head-group g (4 heads).
Each core computes q/k/v projections for its head group (column-sliced
weights), block-causal attention over its batch, and the partial output
projection with its row-slice of wo.  The host sums the 4 per-batch
partials (tensor-parallel partial-sum gather) -- no device collectives.

Device kernel layout notes:
  - All matmul operands are float32r (FP22 multiply, fp32 accumulate).
  - Projections consume host-transposed xT [1024, 2048] so the contraction
    dim (d_model) is on partitions for both operands.
  - Attention computes transposed logits ST = [keys, q] so that P^T is
    directly usable as the moving operand of the AV matmul.
  - The softmax denominator comes for free from a 65th "ones" column on v
    (output row 64 of the AV psum tile).  No max-subtraction: logits are
    ~N(0,1) after the 1/8 scale, far from fp32 exp overflow.
  - Diagonal (partially masked) key blocks are zeroed post-exp with a 0/1
    mask multiplied on the vector engine.
"""
import numpy as np
import ml_dtypes

_bf16 = ml_dtypes.bfloat16

import concourse.bass as bass
import concourse.tile as tile
import concourse.mybir as mybir
from concourse import bacc
from concourse.bass_utils import run_bass_kernel_spmd

B, S, D = 2, 2048, 1024
H, DH = 16, 64
HG = 4                 # heads per core
DG = HG * DH           # 256 projection cols per core
P = 128
QW = 512               # query window (matmul N)
NQW = S // QW          # 4
NKB = S // P           # 16 key blocks
NC = D // P            # 8 contraction chunks of d_model
NRC = S // P           # 16 row chunks
F32 = mybir.dt.float32
F32R = mybir.dt.float32r

_cached_nc = None


def _build_nc():
    nc = bacc.Bacc("TRN2", target_bir_lowering=False, debug=False, num_devices=8)

    BF16 = mybir.dt.bfloat16
    xqT = nc.dram_tensor("xqT", [D, S], BF16, kind="ExternalInput").ap()
    xkT = nc.dram_tensor("xkT", [D, S], BF16, kind="ExternalInput").ap()
    xvT = nc.dram_tensor("xvT", [D, S], BF16, kind="ExternalInput").ap()
    wq = nc.dram_tensor("wq", [D, DG], mybir.dt.bfloat16, kind="ExternalInput").ap()
    wk = nc.dram_tensor("wk", [D, DG], mybir.dt.bfloat16, kind="ExternalInput").ap()
    wv = nc.dram_tensor("wv", [D, DG], mybir.dt.bfloat16, kind="ExternalInput").ap()
    bq = nc.dram_tensor("bq", [DG], F32, kind="ExternalInput").ap()
    bk = nc.dram_tensor("bk", [DG], F32, kind="ExternalInput").ap()
    bv = nc.dram_tensor("bv", [DG], F32R, kind="ExternalInput").ap()
    wo = nc.dram_tensor("wo", [DG, D], F32R, kind="ExternalInput").ap()
    bo = nc.dram_tensor("bo", [D], F32, kind="ExternalInput").ap()
    maskb = nc.dram_tensor("maskb", [P, P], F32R, kind="ExternalInput").ap()
    ident = nc.dram_tensor("ident", [P, P], F32R, kind="ExternalInput").ap()
    onesd = nc.dram_tensor("onesd", [1, P], F32R, kind="ExternalInput").ap()
    out = nc.dram_tensor("out", [S, D], F32, kind="ExternalOutput").ap()

    from contextlib import ExitStack
    with tile.TileContext(nc) as tc, ExitStack() as ctx:
        consts = ctx.enter_context(tc.tile_pool(name="consts", bufs=1))
        slabs = ctx.enter_context(tc.tile_pool(name="slabs", bufs=10))
        persist = ctx.enter_context(tc.tile_pool(name="persist", bufs=1))

        # ---- constants / weights in SBUF (issue order = need order) ----
        wv_sb = consts.tile([P, NC, DG], mybir.dt.bfloat16)
        wv_r = wv.rearrange("(c p) m -> p c m", p=P)
        nc.sync.dma_start(wv_sb[:, 0:1, :], wv_r[:, 0:1, :])
        maskb_sb = consts.tile([P, P], F32R)
        nc.sync.dma_start(maskb_sb[:], maskb)
        ident_sb = consts.tile([P, P], F32R)
        nc.sync.dma_start(ident_sb[:], ident)
        bv_sb = consts.tile([1, DG], F32R)
        ones1 = consts.tile([1, P], F32R)

        # persistent activation storage
        qT = [persist.tile([P, S], F32R, tag=f"qT{t}", name=f"qT{t}") for t in range(2)]
        kT = [persist.tile([P, S], F32R, tag=f"kT{t}", name=f"kT{t}") for t in range(2)]
        v4 = persist.tile([P, NRC, HG, DH + 1], F32R, tag="v4")
        aoT = [persist.tile([P, S], F32R, tag=f"aoT{t}", name=f"aoT{t}") for t in range(2)]
        nc.vector.tensor_scalar(
            out=v4[:, :, :, DH:DH + 1].rearrange("p a b c -> p (a b c)"),
            in0=maskb_sb[:, 0:NRC * HG],
            scalar1=0.0, scalar2=1.0,
            op0=mybir.AluOpType.mult, op1=mybir.AluOpType.add)

        # ---- phase 1: projections (v first, so the q/k psum pool's last
        # users are the q/k evacs and attention isn't gated on v) ----
        with tc.tile_pool(name="ps1", bufs=1, space="PSUM") as ps1:
            # v: rows = seq (16 chunks), cols = head dims (256).
            # One PSUM bank per rowchunk; two passes of 8 rowchunks.
            vslabs = []
            for c in range(NC):
                slab = slabs.tile([P, S], mybir.dt.bfloat16, tag="slab")
                nc.sync.dma_start(slab[:], xvT[c * P:(c + 1) * P, :])
                vslabs.append(slab)
                if c == 0:
                    nc.sync.dma_start(wv_sb[:, 1:NC, :], wv_r[:, 1:NC, :])
                if c == 1:
                    nc.sync.dma_start(bv_sb[:], bv[None, :])
                    nc.sync.dma_start(ones1[:], onesd)
            for vpass in range(2):
                ps = [ps1.tile([P, DG], F32, tag=f"ps1_{i}", name=f"ps1_{i}")
                      for i in range(8)]
                for c in range(NC):
                    for i in range(8):
                        rc = vpass * 8 + i
                        nc.tensor.matmul(
                            ps[i][:],
                            vslabs[c][:, rc * P:(rc + 1) * P],
                            wv_sb[:, c, :],
                            start=(c == 0), stop=False,
                        )
                for i in range(8):
                    rc = vpass * 8 + i
                    nc.tensor.matmul(
                        ps[i][:],
                        ones1[:, :],
                        bv_sb[:, :],
                        start=False, stop=True, skip_group_check=True,
                    )
                    if i % 2 == 0:
                        nc.vector.tensor_copy(
                            out=v4[:, rc, :, 0:DH],
                            in_=ps[i][:].rearrange("p (h d) -> p h d", h=HG),
                        )
                    else:
                        nc.scalar.copy(
                            out=v4[:, rc, :, 0:DH],
                            in_=ps[i][:].rearrange("p (h d) -> p h d", h=HG),
                        )
            # qT then kT: out rows = head dims (2 tiles of 128), cols = seq
            wq_sb = consts.tile([P, NC, DG], mybir.dt.bfloat16)
            nc.sync.dma_start(wq_sb[:], wq.rearrange("(c p) m -> p c m", p=P))
            bq_sb = consts.tile([P, 2], F32)
            nc.sync.dma_start(bq_sb[:], bq.rearrange("(t p) -> p t", p=P))
            wk_sb = consts.tile([P, NC, DG], mybir.dt.bfloat16)
            nc.sync.dma_start(wk_sb[:], wk.rearrange("(c p) m -> p c m", p=P))
            bk_sb = consts.tile([P, 2], F32)
            nc.sync.dma_start(bk_sb[:], bk.rearrange("(t p) -> p t", p=P))
            for name, src, w_sb, b_sb, dst in (
                ("q", xqT, wq_sb, bq_sb, qT),
                ("k", xkT, wk_sb, bk_sb, kT),
            ):
                ps = [ps1.tile([P, QW], F32, tag=f"ps1_{i}", name=f"ps1_{i}") for i in range(8)]
                for c in range(NC):
                    slab = slabs.tile([P, S], mybir.dt.bfloat16, tag="slab")
                    nc.sync.dma_start(slab[:, 0:S // 2],
                                      src[c * P:(c + 1) * P, 0:S // 2])
                    nc.sync.dma_start(slab[:, S // 2:S],
                                      src[c * P:(c + 1) * P, S // 2:S])
                    for t in range(2):
                        for w in range(NQW):
                            nc.tensor.matmul(
                                ps[t * NQW + w][:],
                                w_sb[:, c, t * P:(t + 1) * P],
                                slab[:, w * QW:(w + 1) * QW],
                                start=(c == 0), stop=(c == NC - 1),
                            )
                for t in range(2):
                    for w in range(NQW):
                        if w % 2 == 0:
                            nc.vector.tensor_scalar_add(
                                dst[t][:, w * QW:(w + 1) * QW],
                                ps[t * NQW + w][:],
                                b_sb[:, t:t + 1],
                            )
                        else:
                            nc.scalar.activation(
                                dst[t][:, w * QW:(w + 1) * QW],
                                ps[t * NQW + w][:],
                                mybir.ActivationFunctionType.Identity,
                                bias=b_sb[:, t:t + 1],
                            )

            wo_sb = consts.tile([P, 2, D], F32R)
            nc.sync.dma_start(wo_sb[:], wo.rearrange("(c p) m -> p c m", p=P))
            bo_bc = consts.tile([P, D], F32)
            nc.sync.dma_start(bo_bc[:], bass.AP(
                tensor=bo.tensor, offset=0, ap=[[0, P], [1, D]]))

        # ---- phase 2+3: attention with interleaved output projection ----
        with tc.tile_pool(name="st_ps", bufs=2, space="PSUM") as st_ps, \
             tc.tile_pool(name="ot_ps", bufs=3, space="PSUM") as ot_ps, \
             tc.tile_pool(name="po", bufs=1, space="PSUM") as po, \
             tc.tile_pool(name="ptp", bufs=5) as ptp, \
             tc.tile_pool(name="smp", bufs=4) as smp, \
             tc.tile_pool(name="osb", bufs=4) as osb:
            for qm in range(NQW):
                nkb = 4 * qm + 4
                for hp in range(2):      # head pair = partition halves
                    ot = [ot_ps.tile([DH + 1, QW], F32, tag="ot", name=f"ot{hh}")
                          for hh in range(2)]
                    for kb in range(nkb):
                        st = st_ps.tile([P, 2 * QW], F32, tag="st")
                        joff = kb - 4 * qm
                        # columns below v0 are fully masked for this key
                        # block and never read downstream: skip them in
                        # ST, exp and AV entirely.
                        v0 = max(joff, 0) * P
                        for hh in range(2):
                            lo, hi = hh * DH, (hh + 1) * DH
                            nc.tensor.matmul(
                                st[:, hh * QW + v0:(hh + 1) * QW],
                                kT[hp][lo:hi, kb * P:(kb + 1) * P],
                                qT[hp][lo:hi, qm * QW + v0:(qm + 1) * QW],
                                start=True, stop=(joff < 0),
                            )
                            if joff >= 0:
                                # additive -1e9 triangle on the diagonal
                                # 128-col strip via PE accumulation
                                nc.tensor.matmul(
                                    st[:, hh * QW + v0:hh * QW + v0 + P],
                                    ident_sb[:],
                                    maskb_sb[:],
                                    start=False, stop=True,
                                    skip_group_check=True,
                                )
                        pt = ptp.tile([P, 2 * QW], F32R, tag="pt")
                        if v0 == 0:
                            nc.scalar.activation(
                                pt[:], st[:],
                                mybir.ActivationFunctionType.Exp, scale=0.125)
                        else:
                            for hh in range(2):
                                nc.scalar.activation(
                                    pt[:, hh * QW + v0:(hh + 1) * QW],
                                    st[:, hh * QW + v0:(hh + 1) * QW],
                                    mybir.ActivationFunctionType.Exp,
                                    scale=0.125)
                        for hh in range(2):
                            nc.tensor.matmul(
                                ot[hh][:, v0:QW],
                                v4[:, kb, hp * 2 + hh, :],
                                pt[:, hh * QW + v0:(hh + 1) * QW],
                                start=(kb == 0), stop=(kb == nkb - 1),
                            )
                    for hh in range(2):
                        rcp = smp.tile([1, QW], F32, tag="rcp")
                        nc.vector.reciprocal(rcp[:], ot[hh][DH:DH + 1, :])
                        bc = smp.tile([DH, QW], F32, tag="bc")
                        nc.gpsimd.partition_broadcast(bc[:], rcp[:])
                        nc.vector.tensor_tensor(
                            out=aoT[hp][hh * DH:(hh + 1) * DH,
                                        qm * QW:(qm + 1) * QW],
                            in0=ot[hh][0:DH, :],
                            in1=bc[:],
                            op=mybir.AluOpType.mult,
                        )
                # output projection for this qm's 4 rowchunks
                for rc in range(4 * qm, 4 * qm + 4):
                    o_sb = osb.tile([P, D], F32, tag="o_sb")
                    for nn in range(2):
                        pso = po.tile([P, QW], F32, tag="pso")
                        for hp in range(2):
                            nc.tensor.matmul(
                                pso[:],
                                aoT[hp][:, rc * P:(rc + 1) * P],
                                wo_sb[:, hp, nn * QW:(nn + 1) * QW],
                                start=(hp == 0), stop=(hp == 1),
                            )
                        nc.vector.tensor_tensor(
                            out=o_sb[:, nn * QW:(nn + 1) * QW],
                            in0=pso[:],
                            in1=bo_bc[:, nn * QW:(nn + 1) * QW],
                            op=mybir.AluOpType.add,
                        )
                    nc.sync.dma_start(out[rc * P:(rc + 1) * P, :], o_sb[:])

    nc.compile()
    return nc


def _get_nc():
    global _cached_nc
    if _cached_nc is None:
        _cached_nc = _build_nc()
    return _cached_nc


def _shard_inputs(xk, xq, xv, wq, bq, wk, bk, wv, bv, wo, bo):
    f32 = np.float32
    maskb = np.zeros((P, P), f32)
    for k in range(P):
        maskb[k, :k] = -1.0e9
    ident = np.eye(P, dtype=f32)
    in_maps = []
    for c in range(8):
        b, g = divmod(c, 4)
        gs = slice(g * DG, (g + 1) * DG)
        in_maps.append({
            "xqT": np.ascontiguousarray(np.asarray(xq[b], f32).T.astype(_bf16)),
            "xkT": np.ascontiguousarray(np.asarray(xk[b], f32).T.astype(_bf16)),
            "xvT": np.ascontiguousarray(np.asarray(xv[b], f32).T.astype(_bf16)),
            "wq": np.ascontiguousarray(np.asarray(wq[:, gs], f32).astype(_bf16)),
            "wk": np.ascontiguousarray(np.asarray(wk[:, gs], f32).astype(_bf16)),
            "wv": np.ascontiguousarray(np.asarray(wv[:, gs], f32).astype(_bf16)),
            "bq": np.ascontiguousarray(np.asarray(bq[gs], f32)),
            "bk": np.ascontiguousarray(np.asarray(bk[gs], f32)),
            "bv": np.ascontiguousarray(np.asarray(bv[gs], f32)),
            "wo": np.ascontiguousarray(np.asarray(wo[gs, :], f32)),
            "bo": np.asarray(bo, f32) if g == 0 else np.zeros(D, f32),
            "maskb": maskb,
            "ident": ident,
            "onesd": np.ones((1, P), f32),
        })
    return in_maps


def kernel(xk, xq, xv, wq, bq, wk, bk, wv, bv, wo, bo, _trace=False):
    nc = _get_nc()
    in_maps = _shard_inputs(xk, xq, xv, wq, bq, wk, bk, wv, bv, wo, bo)
    res = run_bass_kernel_spmd(nc, in_maps, core_ids=list(range(8)),
                               trace=_trace)
    parts = [r["out"] for r in res.results]
    out = np.stack([
        parts[0] + parts[1] + parts[2] + parts[3],
        parts[4] + parts[5] + parts[6] + parts[7],
    ]).astype(np.float32)
    if _trace:
        kernel._last_results = res
    return out

